# revision 1
# baseline (speedup 1.0000x reference)
"""NNConv (gnn_message_passing) Bass kernel for 8 Trainium2 NeuronCores.

Strategy (edge-parallel, dst-sharded):
- Host relabels nodes with a permutation so that the 16384 nodes form 128
  "windows" of 128 nodes, each window receiving exactly 512 edges (by
  destination).  Core c owns windows [16c, 16c+16): 2048 nodes / 8192 edges.
  This makes the device program identical across cores (pure SPMD); all
  per-core variation lives in the input data.
- Per 128-edge tile, on device:
    P   = attr_aug^T @ Aaug            (PE matmul -> PSUM, f32)
    q   = relu(P) * x[src] broadcast   (one fused DVE scalar_tensor_tensor,
                                        bf16 out; x gathered by indirect DMA)
    agg += onehot(dst)^T @ q           (PE matmul; zero-step output AP sums
                                        the c_in axis into the same PSUM
                                        columns while contracting edges)
  The root term (x @ root) and a per-window one-hot matmul accumulate into
  the same PSUM bank; bias is added during the PSUM->SBUF copy.
- h1 is exchanged between layers with an AllGather (bf16).
"""

import numpy as np
import ml_dtypes
from contextlib import ExitStack

import concourse.bass as bass
import concourse.tile as tile
from concourse import bacc, mybir
from concourse.bass import IndirectOffsetOnAxis
from concourse.bass_utils import run_bass_kernel_spmd

dt = mybir.dt
BF16 = ml_dtypes.bfloat16

N = 16384
E = 65536
NCORES = 8
P = 128                 # partitions / edges per tile
WINDOWS = 128           # global 128-node windows
WPC = WINDOWS // NCORES  # 16 windows per core
NPC = N // NCORES        # 2048 nodes per core
COUT = 64
CIN1 = 8
CIN2 = 64
ICH = 8                  # c_in values per P-gen chunk (chunk = ICH*COUT cols)

_cached = {}


def _build_program(U):
    """Build the SPMD Bass program. U = edges per window (multiple of 128)."""
    T = U // P  # tiles per window
    EPC = WPC * U  # edges per core

    nc = bacc.Bacc("TRN2", target_bir_lowering=False, debug=False,
                   num_devices=NCORES)

    attrT_d = nc.dram_tensor("attrT", [3, EPC], dt.float32, kind="ExternalInput").ap()
    src_d = nc.dram_tensor("srcidx", [EPC, 1], dt.int32, kind="ExternalInput").ap()
    dstrel_d = nc.dram_tensor("dstrel", [EPC, 1], dt.float32, kind="ExternalInput").ap()
    A1aug_d = nc.dram_tensor("A1aug", [3, CIN1 * COUT], dt.float32, kind="ExternalInput").ap()
    A2aug_d = nc.dram_tensor("A2aug", [3, CIN2 * COUT], dt.float32, kind="ExternalInput").ap()
    xbf_d = nc.dram_tensor("xbf", [N, CIN1], dt.bfloat16, kind="ExternalInput").ap()
    xT_d = nc.dram_tensor("xT", [CIN1, NPC], dt.bfloat16, kind="ExternalInput").ap()
    r1_d = nc.dram_tensor("root1", [CIN1, COUT], dt.bfloat16, kind="ExternalInput").ap()
    r2_d = nc.dram_tensor("root2", [CIN2, COUT], dt.bfloat16, kind="ExternalInput").ap()
    b1_d = nc.dram_tensor("bias1", [P, COUT], dt.float32, kind="ExternalInput").ap()
    b2_d = nc.dram_tensor("bias2", [P, COUT], dt.float32, kind="ExternalInput").ap()
    iota_d = nc.dram_tensor("iota", [P, P], dt.bfloat16, kind="ExternalInput").ap()
    out_d = nc.dram_tensor("out", [NPC, COUT], dt.float32, kind="ExternalOutput").ap()

    with tile.TileContext(nc) as tc, ExitStack() as ctx:
        consts = ctx.enter_context(tc.tile_pool(name="consts", bufs=1))
        meta = ctx.enter_context(tc.tile_pool(name="meta", bufs=4))
        xgp = ctx.enter_context(tc.tile_pool(name="xgp", bufs=4))
        ohp = ctx.enter_context(tc.tile_pool(name="ohp", bufs=4))
        qp = ctx.enter_context(tc.tile_pool(name="qp", bufs=4))
        rootp = ctx.enter_context(tc.tile_pool(name="rootp", bufs=2))
        hp = ctx.enter_context(tc.tile_pool(name="hp", bufs=1))
        outp = ctx.enter_context(tc.tile_pool(name="outp", bufs=3))
        pp = ctx.enter_context(tc.tile_pool(name="pp", bufs=2, space="PSUM"))
        aggp = ctx.enter_context(tc.tile_pool(name="aggp", bufs=3, space="PSUM"))
        dramp = ctx.enter_context(tc.tile_pool(name="dram", bufs=1, space="DRAM"))

        A1_s = consts.tile([3, CIN1 * COUT], dt.float32)
        nc.sync.dma_start(A1_s[:], A1aug_d[:])
        A2_s = consts.tile([3, CIN2 * COUT], dt.float32)
        nc.sync.dma_start(A2_s[:], A2aug_d[:])
        iota_s = consts.tile([P, P], dt.bfloat16)
        nc.sync.dma_start(iota_s[:], iota_d[:])
        r1_s = consts.tile([CIN1, COUT], dt.bfloat16)
        nc.sync.dma_start(r1_s[:], r1_d[:])
        r2_s = consts.tile([CIN2, COUT], dt.bfloat16)
        nc.sync.dma_start(r2_s[:], r2_d[:])
        b1_s = consts.tile([P, COUT], dt.float32)
        nc.sync.dma_start(b1_s[:], b1_d[:])
        b2_s = consts.tile([P, COUT], dt.float32)
        nc.sync.dma_start(b2_s[:], b2_d[:])

        # h1 slice (local) and allgathered h1 (global), bf16
        hloc = dramp.tile([NPC, COUT], dt.bfloat16)
        hglob = dramp.tile([N, COUT], dt.bfloat16)

        def layer(cin, A_s, gather_src, is_l1):
            nchunks = cin // ICH
            for w in range(WPC):
                aggw = aggp.tile([P, COUT], dt.float32, tag="aggw")
                # root-term matmul opens the accumulation (start=True)
                if is_l1:
                    lhsTw = rootp.tile([CIN1, P], dt.bfloat16, tag="rootl1")
                    nc.sync.dma_start(lhsTw[:], xT_d[:, w * P:(w + 1) * P])
                    nc.tensor.matmul(aggw[:], lhsT=lhsTw[:], rhs=r1_s[:],
                                     start=True, stop=False)
                else:
                    lhsTw = rootp.tile([CIN2, P], dt.bfloat16, tag="rootl2")
                    nc.sync.dma_start_transpose(
                        lhsTw[:], hloc[w * P:(w + 1) * P, :])
                    nc.tensor.matmul(aggw[:], lhsT=lhsTw[:], rhs=r2_s[:],
                                     start=True, stop=False)
                for t in range(T):
                    e0 = (w * T + t) * P
                    attr_t = meta.tile([3, P], dt.float32, tag="attr")
                    nc.sync.dma_start(attr_t[:], attrT_d[:, e0:e0 + P])
                    srct = meta.tile([P, 1], dt.int32, tag="src")
                    nc.sync.dma_start(srct[:], src_d[e0:e0 + P, :])
                    dstt = meta.tile([P, 1], dt.float32, tag="dst")
                    nc.sync.dma_start(dstt[:], dstrel_d[e0:e0 + P, :])

                    cin_t = CIN1 if is_l1 else CIN2
                    xg = xgp.tile([P, cin_t], dt.bfloat16,
                                  tag="xg1" if is_l1 else "xg2")
                    nc.gpsimd.indirect_dma_start(
                        out=xg[:], out_offset=None, in_=gather_src,
                        in_offset=IndirectOffsetOnAxis(ap=srct[:, :1], axis=0))

                    oh = ohp.tile([P, P], dt.bfloat16, tag="oh")
                    nc.vector.tensor_scalar(
                        out=oh[:], in0=iota_s[:], scalar1=dstt[:, :1],
                        scalar2=None, op0=mybir.AluOpType.is_equal)

                    for c in range(nchunks):
                        cols = ICH * COUT  # 512
                        ppc = pp.tile([P, cols], dt.float32, tag="ppc")
                        nc.tensor.matmul(
                            ppc[:], lhsT=attr_t[:],
                            rhs=A_s[:, c * cols:(c + 1) * cols],
                            start=True, stop=True)
                        qc = qp.tile([P, cols], dt.bfloat16, tag="qc")
                        q3 = qc[:].rearrange("p (i o) -> p i o", i=ICH)
                        nc.vector.scalar_tensor_tensor(
                            out=q3, in0=ppc[:].rearrange("p (i o) -> p i o", i=ICH),
                            scalar=0.0,
                            in1=xg[:, c * ICH:(c + 1) * ICH].to_broadcast(
                                [P, ICH, COUT]),
                            op0=mybir.AluOpType.max, op1=mybir.AluOpType.mult)
                        nc.tensor.matmul(
                            aggw[:].unsqueeze(1).broadcast_to([P, ICH, COUT]),
                            lhsT=oh[:], rhs=q3,
                            start=False,
                            stop=(t == T - 1 and c == nchunks - 1),
                            skip_group_check=True)
                # finalize window: add bias, write out
                if is_l1:
                    hw_ = outp.tile([P, COUT], dt.bfloat16, tag="h1w")
                    nc.vector.tensor_tensor(out=hw_[:], in0=aggw[:], in1=b1_s[:],
                                            op=mybir.AluOpType.add)
                    nc.sync.dma_start(hloc[w * P:(w + 1) * P, :], hw_[:])
                else:
                    ow = outp.tile([P, COUT], dt.float32, tag="outw")
                    nc.vector.tensor_tensor(out=ow[:], in0=aggw[:], in1=b2_s[:],
                                            op=mybir.AluOpType.add)
                    nc.sync.dma_start(out_d[w * P:(w + 1) * P, :], ow[:])

        layer(CIN1, A1_s, xbf_d[:], True)
        nc.gpsimd.collective_compute(
            "AllGather", mybir.AluOpType.bypass,
            replica_groups=[list(range(NCORES))],
            ins=[hloc[:].opt()], outs=[hglob[:].opt()])
        layer(CIN2, A2_s, hglob[:], False)

    nc.compile()
    return nc


def _pack(edge_index):
    """Relabel nodes into 128 windows of 128 nodes / exactly U edges each.

    Returns (perm, U, order) where perm[orig_node] = new node id and
    order = edge permutation grouping edges by destination window, padded.
    """
    dst = np.asarray(edge_index[1], dtype=np.int64)
    deg = np.bincount(dst, minlength=N).astype(np.int64)
    # LPT greedy: descending degree, least-loaded window with free slots
    nodes = np.argsort(-deg, kind="stable")
    loads = np.zeros(WINDOWS, dtype=np.int64)
    slots = np.zeros(WINDOWS, dtype=np.int64)
    wof = np.empty(N, dtype=np.int64)  # window of node
    for v in nodes:
        open_w = np.flatnonzero(slots < P)
        w = open_w[np.argmin(loads[open_w])]
        wof[v] = w
        loads[w] += deg[v]
        slots[w] += 1
    # repair toward exact target load by swapping nodes between windows
    target = E // WINDOWS
    if loads.max() > target:
        by_wd = {}  # (window, degree) -> list of nodes
        for v in range(N):
            by_wd.setdefault((wof[v], deg[v]), []).append(v)
        for _ in range(100000):
            over = int(np.argmax(loads))
            under = int(np.argmin(loads))
            if loads[over] <= target:
                break
            delta = min(loads[over] - target, target - loads[under])
            # find a swap pair with degree difference = d, largest d first
            done = False
            for d in range(int(delta), 0, -1):
                for da in range(int(deg.max()), d - 1, -1):
                    la = by_wd.get((over, da))
                    lb = by_wd.get((under, da - d))
                    if la and lb:
                        a, b = la.pop(), lb.pop()
                        wof[a], wof[b] = under, over
                        by_wd.setdefault((under, da), []).append(a)
                        by_wd.setdefault((over, da - d), []).append(b)
                        loads[over] -= d
                        loads[under] += d
                        done = True
                        break
                if done:
                    break
            if not done:
                break
    U = int(np.ceil(loads.max() / P) * P)
    # perm: nodes sorted by window -> new ids
    new_order = np.argsort(wof * N + np.arange(N), kind="stable")
    perm = np.empty(N, dtype=np.int64)
    perm[new_order] = np.arange(N)
    # edge order: group by destination window, pad each window to U
    ew = wof[dst]
    eorder = np.argsort(ew, kind="stable")
    counts = np.bincount(ew, minlength=WINDOWS)
    padded = np.full(WINDOWS * U, -1, dtype=np.int64)
    pos = 0
    for w in range(WINDOWS):
        c = int(counts[w])
        padded[w * U:w * U + c] = eorder[pos:pos + c]
        pos += c
    return perm, U, padded


def kernel(x, edge_index, edge_attr, A1, b1, A2, b2, root1, bias1, root2, bias2):
    x = np.asarray(x, dtype=np.float32)
    edge_index = np.asarray(edge_index)
    edge_attr = np.asarray(edge_attr, dtype=np.float32)

    perm, U, padded = _pack(edge_index)
    key = U
    if key not in _cached:
        _cached[key] = _build_program(U)
    nc = _cached[key]

    src = np.asarray(edge_index[0], dtype=np.int64)
    dst = np.asarray(edge_index[1], dtype=np.int64)
    valid = padded >= 0
    pe = np.where(valid, padded, 0)
    # per padded-edge data
    a01 = edge_attr[pe]                      # [W*U, 2]
    aug = valid.astype(np.float32)
    attrT_all = np.stack([a01[:, 0] * aug, a01[:, 1] * aug, aug])  # [3, W*U]
    srcn_all = np.where(valid, perm[src[pe]], 0).astype(np.int32)
    dstn = perm[dst[pe]]
    wof_e = np.arange(WINDOWS).repeat(U)
    dstrel_all = np.where(valid, dstn - wof_e * P, 0).astype(np.float32)

    x_pi = np.empty_like(x)
    x_pi[perm] = x
    x_bf = x_pi.astype(BF16)

    A1aug = np.concatenate([A1, b1[None, :]], axis=0).astype(np.float32)
    A2aug = np.concatenate([A2, b2[None, :]], axis=0).astype(np.float32)
    iota_np = np.broadcast_to(np.arange(P, dtype=np.float32), (P, P)).astype(BF16)
    b1_bc = np.broadcast_to(bias1, (P, COUT)).astype(np.float32).copy()
    b2_bc = np.broadcast_to(bias2, (P, COUT)).astype(np.float32).copy()
    shared = {
        "A1aug": A1aug, "A2aug": A2aug,
        "xbf": np.asarray(x_bf),
        "root1": np.asarray(root1.astype(BF16)),
        "root2": np.asarray(root2.astype(BF16)),
        "bias1": b1_bc, "bias2": b2_bc,
        "iota": np.asarray(iota_np),
    }
    EPC = WPC * U
    in_maps = []
    for c in range(NCORES):
        s = c * EPC
        m = dict(shared)
        m["attrT"] = attrT_all[:, s:s + EPC].copy()
        m["srcidx"] = srcn_all[s:s + EPC].reshape(EPC, 1).copy()
        m["dstrel"] = dstrel_all[s:s + EPC].reshape(EPC, 1).copy()
        m["xT"] = np.ascontiguousarray(x_bf[c * NPC:(c + 1) * NPC].T)
        in_maps.append(m)

    res = run_bass_kernel_spmd(nc, in_maps, list(range(NCORES)),
                               **kernel.run_kwargs)
    kernel.last_result = res
    out_pi = np.concatenate([res.results[c]["out"] for c in range(NCORES)], axis=0)
    return out_pi[perm]


kernel.run_kwargs = {}
kernel.last_result = None



# revision 5
# speedup vs baseline: 2.3048x; 2.3048x over previous
"""NNConv (gnn_message_passing) Bass kernel for 8 Trainium2 NeuronCores.

Strategy (edge-parallel, dst-sharded):
- Host relabels nodes with a permutation so that the 16384 nodes form 128
  "windows" of 128 nodes, each window receiving exactly U edges (by
  destination).  Core c owns windows [16c, 16c+16): 2048 nodes / 8192 edges.
  Pure SPMD; per-core variation lives only in the input data.
- Per 128-edge tile, on device (all matmuls bf16):
    P   = attr_aug^T @ Aaug            (PE matmul -> PSUM f32, K=3)
    q   = relu(P) * x[src] broadcast   (one DVE scalar_tensor_tensor per
                                        1024-col unit, bf16 out)
    agg += onehot(dst)^T @ q           (PE matmul; zero-step output AP sums
                                        the c_in axis while contracting edges)
  The scatter matmuls for tile t are emitted during tile t+1's generation so
  the PE never stalls waiting for the DVE.  The root term (x @ root) and the
  node bias ride in a single augmented matmul that opens each window's PSUM
  accumulation.
- h1 is exchanged between layers with an AllGather (bf16).
"""

import numpy as np
import ml_dtypes
from contextlib import ExitStack

import concourse.bass as bass
import concourse.tile as tile
from concourse import bacc, mybir
from concourse.bass import IndirectOffsetOnAxis
from concourse.bass_utils import run_bass_kernel_spmd

dt = mybir.dt
BF16 = ml_dtypes.bfloat16

N = 16384
E = 65536
NCORES = 8
P = 128                 # partitions / edges per tile
WINDOWS = 128           # global 128-node windows
WPC = WINDOWS // NCORES  # 16 windows per core
NPC = N // NCORES        # 2048 nodes per core
COUT = 64
CIN1 = 8
CIN2 = 64

_cached = {}


def _build_program(U):
    """Build the SPMD Bass program. U = edges per window (multiple of 128)."""
    T = U // P    # tiles per window
    NT = WPC * T  # tiles per core per layer
    EPC = WPC * U

    nc = bacc.Bacc("TRN2", target_bir_lowering=False, debug=False,
                   num_devices=NCORES)

    attrT_d = nc.dram_tensor("attrT", [3, EPC], dt.bfloat16, kind="ExternalInput").ap()
    srcw_d = nc.dram_tensor("srcw", [P, NT], dt.int32, kind="ExternalInput").ap()
    dstw_d = nc.dram_tensor("dstw", [P, NT], dt.float32, kind="ExternalInput").ap()
    A1_d = nc.dram_tensor("A1aug", [3, CIN1 * COUT], dt.bfloat16, kind="ExternalInput").ap()
    A2_d = nc.dram_tensor("A2aug", [3, CIN2 * COUT], dt.bfloat16, kind="ExternalInput").ap()
    xbf_d = nc.dram_tensor("xbf", [N, CIN1], dt.bfloat16, kind="ExternalInput").ap()
    xT_d = nc.dram_tensor("xT9", [CIN1 + 1, NPC], dt.bfloat16, kind="ExternalInput").ap()
    r1_d = nc.dram_tensor("r1aug", [CIN1 + 1, COUT], dt.bfloat16, kind="ExternalInput").ap()
    r2_d = nc.dram_tensor("r2aug", [CIN2 + 1, COUT], dt.bfloat16, kind="ExternalInput").ap()
    iota_d = nc.dram_tensor("iota", [P, P], dt.bfloat16, kind="ExternalInput").ap()
    out_d = nc.dram_tensor("out", [NPC, COUT], dt.float32, kind="ExternalOutput").ap()

    with tile.TileContext(nc) as tc, ExitStack() as ctx:
        consts = ctx.enter_context(tc.tile_pool(name="consts", bufs=1))
        wmeta = ctx.enter_context(tc.tile_pool(name="wmeta", bufs=3))
        xgp = ctx.enter_context(tc.tile_pool(name="xgp", bufs=4))
        ohp = ctx.enter_context(tc.tile_pool(name="ohp", bufs=4))
        qp = ctx.enter_context(tc.tile_pool(name="qp", bufs=6))
        rootp = ctx.enter_context(tc.tile_pool(name="rootp", bufs=2))
        outp = ctx.enter_context(tc.tile_pool(name="outp", bufs=3))
        pp = ctx.enter_context(tc.tile_pool(name="pp", bufs=3, space="PSUM"))
        aggp = ctx.enter_context(tc.tile_pool(name="aggp", bufs=2, space="PSUM"))
        dramp = ctx.enter_context(tc.tile_pool(name="dram", bufs=1, space="DRAM"))

        A1_s = consts.tile([3, CIN1 * COUT], dt.bfloat16)
        nc.sync.dma_start(A1_s[:], A1_d[:])
        A2_s = consts.tile([3, CIN2 * COUT], dt.bfloat16)
        nc.sync.dma_start(A2_s[:], A2_d[:])
        iota_s = consts.tile([P, P], dt.bfloat16)
        nc.sync.dma_start(iota_s[:], iota_d[:])
        r1_s = consts.tile([CIN1 + 1, COUT], dt.bfloat16)
        nc.sync.dma_start(r1_s[:], r1_d[:])
        r2_s = consts.tile([CIN2 + 1, COUT], dt.bfloat16)
        nc.sync.dma_start(r2_s[:], r2_d[:])
        # ping-pong lhsT buffers for the layer-2 root matmul: rows 0:64 get
        # h1^T via transpose-DMA each window, row 64 stays all-ones.
        h1T = [consts.tile([CIN2 + 1, P], dt.bfloat16, name=f"h1T{i}",
                           tag=f"h1T{i}")
               for i in range(2)]
        for hT in h1T:
            nc.vector.tensor_scalar(
                out=hT[CIN2:CIN2 + 1, :], in0=iota_s[0:1, :], scalar1=-1.0,
                scalar2=None, op0=mybir.AluOpType.is_ge)

        # h1 slice (local) and allgathered h1 (global), bf16
        hloc = dramp.tile([NPC, COUT], dt.bfloat16)
        hglob = dramp.tile([N, COUT], dt.bfloat16)

        def layer(is_l1):
            cin = CIN1 if is_l1 else CIN2
            cols = cin * COUT           # 512 or 4096
            A_s = A1_s if is_l1 else A2_s
            gather_src = xbf_d[:] if is_l1 else hglob[:]
            nunits = max(1, cols // 1024)
            ucols = min(cols, 1024)
            uich = ucols // COUT        # 8 (L1) or 16 (L2)

            state = {"attr_w": None, "src_w": None, "dst_w": None}
            aggws = {}
            pending = None  # (oh, qts, w, is_last_tile_of_window)

            def emit_scat(p):
                oh, qts, w, last = p
                aggw = aggws[w]
                nmm = sum(qt.shape[1] // 512 for qt in qts)
                k = 0
                for qt in qts:
                    for h in range(qt.shape[1] // 512):
                        k += 1
                        q3 = qt[:, h * 512:(h + 1) * 512].rearrange(
                            "p (i o) -> p i o", i=8)
                        nc.tensor.matmul(
                            aggw[:].unsqueeze(1).broadcast_to([P, 8, COUT]),
                            lhsT=oh[:], rhs=q3,
                            start=False, stop=(last and k == nmm),
                            skip_group_check=True)
                if last:
                    # finalize window: copy PSUM -> SBUF, write out
                    if is_l1:
                        hw_ = outp.tile([P, COUT], dt.bfloat16, tag="h1w")
                        nc.vector.tensor_copy(hw_[:], aggw[:])
                        nc.sync.dma_start(hloc[w * P:(w + 1) * P, :], hw_[:])
                    else:
                        ow = outp.tile([P, COUT], dt.float32, tag="outw")
                        nc.vector.tensor_copy(ow[:], aggw[:])
                        nc.sync.dma_start(out_d[w * P:(w + 1) * P, :], ow[:])

            for g in range(NT):
                w, t = divmod(g, T)
                if t == 0:
                    attr_w = wmeta.tile([3, U], dt.bfloat16, tag="attrw")
                    nc.sync.dma_start(attr_w[:], attrT_d[:, w * U:(w + 1) * U])
                    src_w = wmeta.tile([P, T], dt.int32, tag="srcw")
                    nc.sync.dma_start(src_w[:], srcw_d[:, w * T:(w + 1) * T])
                    dst_w = wmeta.tile([P, T], dt.float32, tag="dstw")
                    nc.sync.dma_start(dst_w[:], dstw_d[:, w * T:(w + 1) * T])
                    state.update(attr_w=attr_w, src_w=src_w, dst_w=dst_w)
                    aggw = aggp.tile([P, COUT], dt.float32, tag="aggw")
                    aggws[w] = aggw
                    # root matmul (with bias folded in) opens the accumulation
                    if is_l1:
                        lhsTw = rootp.tile([CIN1 + 1, P], dt.bfloat16, tag="rootl1")
                        nc.sync.dma_start(lhsTw[:], xT_d[:, w * P:(w + 1) * P])
                        nc.tensor.matmul(aggw[:], lhsT=lhsTw[:], rhs=r1_s[:],
                                         start=True, stop=False)
                    else:
                        hT = h1T[w % 2]
                        nc.sync.dma_start_transpose(
                            hT[0:CIN2, :], hloc[w * P:(w + 1) * P, :])
                        nc.tensor.matmul(aggw[:], lhsT=hT[:], rhs=r2_s[:],
                                         start=True, stop=False)
                else:
                    attr_w, src_w, dst_w = (state["attr_w"], state["src_w"],
                                            state["dst_w"])

                xg = xgp.tile([P, cin], dt.bfloat16,
                              tag="xg1" if is_l1 else "xg2")
                nc.gpsimd.indirect_dma_start(
                    out=xg[:], out_offset=None, in_=gather_src,
                    in_offset=IndirectOffsetOnAxis(ap=src_w[:, t:t + 1], axis=0))

                oh = ohp.tile([P, P], dt.bfloat16, tag="oh")
                nc.vector.tensor_scalar(
                    out=oh[:], in0=iota_s[:], scalar1=dst_w[:, t:t + 1],
                    scalar2=None, op0=mybir.AluOpType.is_equal)

                attr_t = attr_w[:, t * P:(t + 1) * P]
                qts = []
                for u in range(nunits):
                    pu = pp.tile([P, ucols], dt.float32, name="pu", tag="pu",
                                 padded_shape=[P, 1024])
                    for h in range(ucols // 512):
                        c0 = u * ucols + h * 512
                        nc.tensor.matmul(
                            pu[:, h * 512:(h + 1) * 512], lhsT=attr_t,
                            rhs=A_s[:, c0:c0 + 512], start=True, stop=True)
                    qt = qp.tile([P, ucols], dt.bfloat16,
                                 tag="q1" if is_l1 else "q2")
                    nc.vector.scalar_tensor_tensor(
                        out=qt[:].rearrange("p (i o) -> p i o", i=uich),
                        in0=pu[:].rearrange("p (i o) -> p i o", i=uich),
                        scalar=0.0,
                        in1=xg[:, u * uich:(u + 1) * uich].to_broadcast(
                            [P, uich, COUT]),
                        op0=mybir.AluOpType.max, op1=mybir.AluOpType.mult)
                    qts.append(qt)

                if pending is not None:
                    emit_scat(pending)
                pending = (oh, qts, w, t == T - 1)
            emit_scat(pending)

        layer(True)
        nc.gpsimd.collective_compute(
            "AllGather", mybir.AluOpType.bypass,
            replica_groups=[list(range(NCORES))],
            ins=[hloc[:].opt()], outs=[hglob[:].opt()])
        layer(False)

    nc.compile()
    return nc


def _pack(edge_index):
    """Relabel nodes into 128 windows of 128 nodes / exactly U edges each.

    Returns (perm, U, order) where perm[orig_node] = new node id and
    order = edge permutation grouping edges by destination window, padded.
    """
    dst = np.asarray(edge_index[1], dtype=np.int64)
    deg = np.bincount(dst, minlength=N).astype(np.int64)
    # LPT greedy: descending degree, least-loaded window with free slots
    nodes = np.argsort(-deg, kind="stable")
    loads = np.zeros(WINDOWS, dtype=np.int64)
    slots = np.zeros(WINDOWS, dtype=np.int64)
    wof = np.empty(N, dtype=np.int64)  # window of node
    for v in nodes:
        open_w = np.flatnonzero(slots < P)
        w = open_w[np.argmin(loads[open_w])]
        wof[v] = w
        loads[w] += deg[v]
        slots[w] += 1
    # repair toward exact target load by swapping nodes between windows
    target = E // WINDOWS
    if loads.max() > target:
        by_wd = {}  # (window, degree) -> list of nodes
        for v in range(N):
            by_wd.setdefault((wof[v], deg[v]), []).append(v)
        for _ in range(100000):
            over = int(np.argmax(loads))
            under = int(np.argmin(loads))
            if loads[over] <= target:
                break
            delta = min(loads[over] - target, target - loads[under])
            # find a swap pair with degree difference = d, largest d first
            done = False
            for d in range(int(delta), 0, -1):
                for da in range(int(deg.max()), d - 1, -1):
                    la = by_wd.get((over, da))
                    lb = by_wd.get((under, da - d))
                    if la and lb:
                        a, b = la.pop(), lb.pop()
                        wof[a], wof[b] = under, over
                        by_wd.setdefault((under, da), []).append(a)
                        by_wd.setdefault((over, da - d), []).append(b)
                        loads[over] -= d
                        loads[under] += d
                        done = True
                        break
                if done:
                    break
            if not done:
                break
    U = int(np.ceil(loads.max() / P) * P)
    # perm: nodes sorted by window -> new ids
    new_order = np.argsort(wof * N + np.arange(N), kind="stable")
    perm = np.empty(N, dtype=np.int64)
    perm[new_order] = np.arange(N)
    # edge order: group by destination window, pad each window to U
    ew = wof[dst]
    eorder = np.argsort(ew, kind="stable")
    counts = np.bincount(ew, minlength=WINDOWS)
    padded = np.full(WINDOWS * U, -1, dtype=np.int64)
    pos = 0
    for w in range(WINDOWS):
        c = int(counts[w])
        padded[w * U:w * U + c] = eorder[pos:pos + c]
        pos += c
    return perm, U, padded


def kernel(x, edge_index, edge_attr, A1, b1, A2, b2, root1, bias1, root2, bias2):
    x = np.asarray(x, dtype=np.float32)
    edge_index = np.asarray(edge_index)
    edge_attr = np.asarray(edge_attr, dtype=np.float32)

    perm, U, padded = _pack(edge_index)
    T = U // P
    NT = WPC * T
    key = U
    if key not in _cached:
        _cached[key] = _build_program(U)
    nc = _cached[key]

    src = np.asarray(edge_index[0], dtype=np.int64)
    dst = np.asarray(edge_index[1], dtype=np.int64)
    valid = padded >= 0
    pe = np.where(valid, padded, 0)
    # per padded-edge data
    a01 = edge_attr[pe]                      # [W*U, 2]
    aug = valid.astype(np.float32)
    attrT_all = np.stack([a01[:, 0] * aug, a01[:, 1] * aug, aug]).astype(BF16)
    srcn_all = np.where(valid, perm[src[pe]], 0).astype(np.int32)
    dstn = perm[dst[pe]]
    wof_e = np.arange(WINDOWS).repeat(U)
    dstrel_all = np.where(valid, dstn - wof_e * P, 0).astype(np.float32)

    x_pi = np.empty_like(x)
    x_pi[perm] = x
    x_bf = x_pi.astype(BF16)

    A1aug = np.concatenate([A1, b1[None, :]], axis=0).astype(BF16)
    A2aug = np.concatenate([A2, b2[None, :]], axis=0).astype(BF16)
    r1aug = np.concatenate([root1, bias1[None, :]], axis=0).astype(BF16)
    r2aug = np.concatenate([root2, bias2[None, :]], axis=0).astype(BF16)
    iota_np = np.broadcast_to(np.arange(P, dtype=np.float32), (P, P)).astype(BF16)
    shared = {
        "A1aug": np.asarray(A1aug), "A2aug": np.asarray(A2aug),
        "xbf": np.asarray(x_bf),
        "r1aug": np.asarray(r1aug), "r2aug": np.asarray(r2aug),
        "iota": np.asarray(iota_np),
    }
    EPC = WPC * U
    in_maps = []
    ones9 = np.ones((1, NPC), dtype=BF16)
    for c in range(NCORES):
        s = c * EPC
        m = dict(shared)
        m["attrT"] = attrT_all[:, s:s + EPC].copy()
        # [P, NT] with column (w*T + t) = edges [wU + t*128 : wU + (t+1)*128)
        m["srcw"] = np.ascontiguousarray(
            srcn_all[s:s + EPC].reshape(NT, P).T)
        m["dstw"] = np.ascontiguousarray(
            dstrel_all[s:s + EPC].reshape(NT, P).T)
        xTc = np.ascontiguousarray(x_bf[c * NPC:(c + 1) * NPC].T)
        m["xT9"] = np.concatenate([xTc, ones9], axis=0)
        in_maps.append(m)

    res = run_bass_kernel_spmd(nc, in_maps, list(range(NCORES)),
                               **kernel.run_kwargs)
    kernel.last_result = res
    out_pi = np.concatenate([res.results[c]["out"] for c in range(NCORES)], axis=0)
    return out_pi[perm]


kernel.run_kwargs = {}
kernel.last_result = None


# revision 11
# speedup vs baseline: 2.7360x; 1.1871x over previous
"""NNConv (gnn_message_passing) Bass kernel for 8 Trainium2 NeuronCores.

Strategy (edge-parallel, dst-sharded):
- Host relabels nodes with a permutation so that the 16384 nodes form 128
  "windows" of 128 nodes, each window receiving exactly U edges (by
  destination).  Core c owns windows [16c, 16c+16): 2048 nodes / 8192 edges.
  Pure SPMD; per-core variation lives only in the input data.
- Per 128-edge tile, on device (all matmuls bf16):
    P   = attr_aug^T @ Aaug            (PE matmul -> PSUM f32, K=3)
    q   = relu(P) * x[src] broadcast   (one DVE scalar_tensor_tensor per
                                        1024-col unit, bf16 out)
    agg += onehot(dst)^T @ q           (PE matmul; zero-step output AP sums
                                        the c_in axis while contracting edges)
  The scatter matmuls for tile t are emitted during tile t+1's generation so
  the PE never stalls waiting for the DVE.  The root term (x @ root) and the
  node bias ride in a single augmented matmul that opens each window's PSUM
  accumulation.
- h1 is exchanged between layers with an AllGather (bf16).
"""

import numpy as np
import ml_dtypes
from contextlib import ExitStack

import concourse.bass as bass
import concourse.tile as tile
from concourse import bacc, mybir
from concourse.bass import IndirectOffsetOnAxis
from concourse.bass_utils import run_bass_kernel_spmd

dt = mybir.dt
BF16 = ml_dtypes.bfloat16

N = 16384
E = 65536
NCORES = 8
P = 128                 # partitions / edges per tile
WINDOWS = 128           # global 128-node windows
WPC = WINDOWS // NCORES  # 16 windows per core
NPC = N // NCORES        # 2048 nodes per core
COUT = 64
CIN1 = 8
CIN2 = 64

_cached = {}


def _build_program(U):
    """Build the SPMD Bass program. U = edges per window (multiple of 128)."""
    T = U // P    # tiles per window
    NT = WPC * T  # tiles per core per layer
    EPC = WPC * U

    nc = bacc.Bacc("TRN2", target_bir_lowering=False, debug=False,
                   num_devices=NCORES)

    # attr and A replicated at partition quadrants 0/32/64/96 so four K=3
    # generator matmuls can run concurrently in distinct PE row groups.
    attrT_d = nc.dram_tensor("attrT", [99, EPC], dt.bfloat16, kind="ExternalInput").ap()
    srcw_d = nc.dram_tensor("srcw", [P, NT], dt.int32, kind="ExternalInput").ap()
    dstw_d = nc.dram_tensor("dstw", [P, NT], dt.float32, kind="ExternalInput").ap()
    A1_d = nc.dram_tensor("A1aug", [99, CIN1 * COUT], dt.bfloat16, kind="ExternalInput").ap()
    A2_d = nc.dram_tensor("A2aug", [99, CIN2 * COUT], dt.bfloat16, kind="ExternalInput").ap()
    xbf_d = nc.dram_tensor("xbf", [N, CIN1], dt.bfloat16, kind="ExternalInput").ap()
    xT_d = nc.dram_tensor("xT9", [CIN1 + 1, NPC], dt.bfloat16, kind="ExternalInput").ap()
    r1_d = nc.dram_tensor("r1aug", [CIN1 + 1, COUT], dt.bfloat16, kind="ExternalInput").ap()
    r2_d = nc.dram_tensor("r2aug", [CIN2 + 1, COUT], dt.bfloat16, kind="ExternalInput").ap()
    iota_d = nc.dram_tensor("iota", [P, P], dt.bfloat16, kind="ExternalInput").ap()
    out_d = nc.dram_tensor("out", [NPC, COUT], dt.float32, kind="ExternalOutput").ap()

    with tile.TileContext(nc) as tc, ExitStack() as ctx:
        consts = ctx.enter_context(tc.tile_pool(name="consts", bufs=1))
        wmeta = ctx.enter_context(tc.tile_pool(name="wmeta", bufs=3))
        xgp = ctx.enter_context(tc.tile_pool(name="xgp", bufs=4))
        ohp = ctx.enter_context(tc.tile_pool(name="ohp", bufs=4))
        qp = ctx.enter_context(tc.tile_pool(name="qp", bufs=6))
        rootp = ctx.enter_context(tc.tile_pool(name="rootp", bufs=2))
        outp = ctx.enter_context(tc.tile_pool(name="outp", bufs=3))
        pp = ctx.enter_context(tc.tile_pool(name="pp", bufs=3, space="PSUM"))
        aggp = ctx.enter_context(tc.tile_pool(name="aggp", bufs=2, space="PSUM"))
        dramp = ctx.enter_context(tc.tile_pool(name="dram", bufs=1, space="DRAM"))

        A1_s = consts.tile([99, CIN1 * COUT], dt.bfloat16)
        nc.sync.dma_start(A1_s[:], A1_d[:])
        A2_s = consts.tile([99, CIN2 * COUT], dt.bfloat16)
        nc.sync.dma_start(A2_s[:], A2_d[:])
        iota_s = consts.tile([P, P], dt.bfloat16)
        nc.sync.dma_start(iota_s[:], iota_d[:])
        r1_s = consts.tile([CIN1 + 1, COUT], dt.bfloat16)
        nc.sync.dma_start(r1_s[:], r1_d[:])
        r2_s = consts.tile([CIN2 + 1, COUT], dt.bfloat16)
        nc.sync.dma_start(r2_s[:], r2_d[:])
        # ping-pong lhsT buffers for the layer-2 root matmul: rows 0:64 get
        # h1^T via transpose-DMA each window, row 64 stays all-ones.
        h1T = [consts.tile([CIN2 + 1, P], dt.bfloat16, name=f"h1T{i}",
                           tag=f"h1T{i}")
               for i in range(2)]
        for hT in h1T:
            nc.vector.tensor_scalar(
                out=hT[CIN2:CIN2 + 1, :], in0=iota_s[0:1, :], scalar1=-1.0,
                scalar2=None, op0=mybir.AluOpType.is_ge)

        # h1 slice (local) and allgathered h1 (global), bf16
        hloc = dramp.tile([NPC, COUT], dt.bfloat16)
        hglob = dramp.tile([N, COUT], dt.bfloat16)

        def layer(is_l1):
            cin = CIN1 if is_l1 else CIN2
            cols = cin * COUT           # 512 or 4096
            A_s = A1_s if is_l1 else A2_s
            gather_src = xbf_d[:] if is_l1 else hglob[:]
            nunits = max(1, cols // 1024)
            ucols = min(cols, 1024)
            uich = ucols // COUT        # 8 (L1) or 16 (L2)

            state = {"attr_w": None, "src_w": None, "dst_w": None}
            aggws = {}
            pending = None  # (oh, qts, w, is_last_tile_of_window)

            def emit_scat(p):
                oh, qts, w, last = p
                aggw = aggws[w]
                nmm = sum(qt.shape[1] // 512 for qt in qts)
                k = 0
                for qt in qts:
                    for h in range(qt.shape[1] // 512):
                        k += 1
                        q3 = qt[:, h * 512:(h + 1) * 512].rearrange(
                            "p (i o) -> p i o", i=8)
                        nc.tensor.matmul(
                            aggw[:].unsqueeze(1).broadcast_to([P, 8, COUT]),
                            lhsT=oh[:], rhs=q3,
                            start=False, stop=(last and k == nmm),
                            skip_group_check=True)
                if last:
                    # finalize window: copy PSUM -> SBUF, write out
                    if is_l1:
                        hw_ = outp.tile([P, COUT], dt.bfloat16, tag="h1w")
                        nc.vector.tensor_copy(hw_[:], aggw[:])
                        nc.sync.dma_start(hloc[w * P:(w + 1) * P, :], hw_[:])
                    else:
                        ow = outp.tile([P, COUT], dt.float32, tag="outw")
                        nc.vector.tensor_copy(ow[:], aggw[:])
                        nc.sync.dma_start(out_d[w * P:(w + 1) * P, :], ow[:])

            for g in range(NT):
                w, t = divmod(g, T)
                if t == 0:
                    attr_w = wmeta.tile([99, U], dt.bfloat16, tag="attrw")
                    nc.sync.dma_start(attr_w[:], attrT_d[:, w * U:(w + 1) * U])
                    src_w = wmeta.tile([P, T], dt.int32, tag="srcw")
                    nc.sync.dma_start(src_w[:], srcw_d[:, w * T:(w + 1) * T])
                    dst_w = wmeta.tile([P, T], dt.float32, tag="dstw")
                    nc.sync.dma_start(dst_w[:], dstw_d[:, w * T:(w + 1) * T])
                    state.update(attr_w=attr_w, src_w=src_w, dst_w=dst_w)
                    aggw = aggp.tile([P, COUT], dt.float32, tag="aggw")
                    aggws[w] = aggw
                    # root matmul (with bias folded in) opens the accumulation
                    if is_l1:
                        lhsTw = rootp.tile([CIN1 + 1, P], dt.bfloat16, tag="rootl1")
                        nc.sync.dma_start(lhsTw[:], xT_d[:, w * P:(w + 1) * P])
                        nc.tensor.matmul(aggw[:], lhsT=lhsTw[:], rhs=r1_s[:],
                                         start=True, stop=False)
                    else:
                        hT = h1T[w % 2]
                        nc.sync.dma_start_transpose(
                            hT[0:CIN2, :], hloc[w * P:(w + 1) * P, :])
                        nc.tensor.matmul(aggw[:], lhsT=hT[:], rhs=r2_s[:],
                                         start=True, stop=False)
                else:
                    attr_w, src_w, dst_w = (state["attr_w"], state["src_w"],
                                            state["dst_w"])

                xg = xgp.tile([P, cin], dt.bfloat16,
                              tag="xg1" if is_l1 else "xg2")
                nc.gpsimd.indirect_dma_start(
                    out=xg[:], out_offset=None, in_=gather_src,
                    in_offset=IndirectOffsetOnAxis(ap=src_w[:, t:t + 1], axis=0))

                oh = ohp.tile([P, P], dt.bfloat16, tag="oh")
                nc.vector.tensor_scalar(
                    out=oh[:], in0=iota_s[:], scalar1=dst_w[:, t:t + 1],
                    scalar2=None, op0=mybir.AluOpType.is_equal)

                qts = []
                for u in range(nunits):
                    pu = pp.tile([P, ucols], dt.float32, name="pu", tag="pu",
                                 padded_shape=[P, 1024])
                    for h in range(ucols // 512):
                        c0 = u * ucols + h * 512
                        # spread the K=3 matmuls over the 4 PE row groups
                        rg = 32 * ((u * (ucols // 512) + h) % 4) if cols >= 2048 \
                            else 32 * (g % 4)
                        nc.tensor.matmul(
                            pu[:, h * 512:(h + 1) * 512],
                            lhsT=attr_w[rg:rg + 3, t * P:(t + 1) * P],
                            rhs=A_s[rg:rg + 3, c0:c0 + 512],
                            start=True, stop=True, tile_position=(rg, 0))
                    qt = qp.tile([P, ucols], dt.bfloat16,
                                 tag="q1" if is_l1 else "q2")
                    nc.vector.scalar_tensor_tensor(
                        out=qt[:].rearrange("p (i o) -> p i o", i=uich),
                        in0=pu[:].rearrange("p (i o) -> p i o", i=uich),
                        scalar=0.0,
                        in1=xg[:, u * uich:(u + 1) * uich].to_broadcast(
                            [P, uich, COUT]),
                        op0=mybir.AluOpType.max, op1=mybir.AluOpType.mult)
                    qts.append(qt)

                if pending is not None:
                    emit_scat(pending)
                pending = (oh, qts, w, t == T - 1)
            emit_scat(pending)

        layer(True)
        nc.gpsimd.collective_compute(
            "AllGather", mybir.AluOpType.bypass,
            replica_groups=[list(range(NCORES))],
            ins=[hloc[:].opt()], outs=[hglob[:].opt()])
        layer(False)

    nc.compile()
    return nc


def _pack(edge_index):
    """Relabel nodes into 128 windows of 128 nodes / exactly U edges each.

    Returns (perm, U, order) where perm[orig_node] = new node id and
    order = edge permutation grouping edges by destination window, padded.
    """
    dst = np.asarray(edge_index[1], dtype=np.int64)
    deg = np.bincount(dst, minlength=N).astype(np.int64)
    # LPT greedy: descending degree, least-loaded window with free slots
    nodes = np.argsort(-deg, kind="stable")
    loads = np.zeros(WINDOWS, dtype=np.int64)
    slots = np.zeros(WINDOWS, dtype=np.int64)
    wof = np.empty(N, dtype=np.int64)  # window of node
    for v in nodes:
        open_w = np.flatnonzero(slots < P)
        w = open_w[np.argmin(loads[open_w])]
        wof[v] = w
        loads[w] += deg[v]
        slots[w] += 1
    # repair toward exact target load by swapping nodes between windows
    target = E // WINDOWS
    if loads.max() > target:
        by_wd = {}  # (window, degree) -> list of nodes
        for v in range(N):
            by_wd.setdefault((wof[v], deg[v]), []).append(v)
        for _ in range(100000):
            over = int(np.argmax(loads))
            under = int(np.argmin(loads))
            if loads[over] <= target:
                break
            delta = min(loads[over] - target, target - loads[under])
            # find a swap pair with degree difference = d, largest d first
            done = False
            for d in range(int(delta), 0, -1):
                for da in range(int(deg.max()), d - 1, -1):
                    la = by_wd.get((over, da))
                    lb = by_wd.get((under, da - d))
                    if la and lb:
                        a, b = la.pop(), lb.pop()
                        wof[a], wof[b] = under, over
                        by_wd.setdefault((under, da), []).append(a)
                        by_wd.setdefault((over, da - d), []).append(b)
                        loads[over] -= d
                        loads[under] += d
                        done = True
                        break
                if done:
                    break
            if not done:
                break
    U = int(np.ceil(loads.max() / P) * P)
    # perm: nodes sorted by window -> new ids
    new_order = np.argsort(wof * N + np.arange(N), kind="stable")
    perm = np.empty(N, dtype=np.int64)
    perm[new_order] = np.arange(N)
    # edge order: group by destination window, pad each window to U
    ew = wof[dst]
    eorder = np.argsort(ew, kind="stable")
    counts = np.bincount(ew, minlength=WINDOWS)
    padded = np.full(WINDOWS * U, -1, dtype=np.int64)
    pos = 0
    for w in range(WINDOWS):
        c = int(counts[w])
        padded[w * U:w * U + c] = eorder[pos:pos + c]
        pos += c
    return perm, U, padded


def kernel(x, edge_index, edge_attr, A1, b1, A2, b2, root1, bias1, root2, bias2):
    x = np.asarray(x, dtype=np.float32)
    edge_index = np.asarray(edge_index)
    edge_attr = np.asarray(edge_attr, dtype=np.float32)

    perm, U, padded = _pack(edge_index)
    T = U // P
    NT = WPC * T
    key = U
    if key not in _cached:
        _cached[key] = _build_program(U)
    nc = _cached[key]

    src = np.asarray(edge_index[0], dtype=np.int64)
    dst = np.asarray(edge_index[1], dtype=np.int64)
    valid = padded >= 0
    pe = np.where(valid, padded, 0)
    # per padded-edge data
    a01 = edge_attr[pe]                      # [W*U, 2]
    aug = valid.astype(np.float32)
    attr3 = np.stack([a01[:, 0] * aug, a01[:, 1] * aug, aug]).astype(BF16)
    attrT_all = np.zeros((99, attr3.shape[1]), dtype=BF16)
    for rg in range(4):
        attrT_all[32 * rg:32 * rg + 3] = attr3
    srcn_all = np.where(valid, perm[src[pe]], 0).astype(np.int32)
    dstn = perm[dst[pe]]
    wof_e = np.arange(WINDOWS).repeat(U)
    dstrel_all = np.where(valid, dstn - wof_e * P, 0).astype(np.float32)

    x_pi = np.empty_like(x)
    x_pi[perm] = x
    x_bf = x_pi.astype(BF16)

    def rep4(Aaug3):
        out = np.zeros((99, Aaug3.shape[1]), dtype=BF16)
        for rg in range(4):
            out[32 * rg:32 * rg + 3] = Aaug3
        return out
    A1aug = rep4(np.concatenate([A1, b1[None, :]], axis=0).astype(BF16))
    A2aug = rep4(np.concatenate([A2, b2[None, :]], axis=0).astype(BF16))
    r1aug = np.concatenate([root1, bias1[None, :]], axis=0).astype(BF16)
    r2aug = np.concatenate([root2, bias2[None, :]], axis=0).astype(BF16)
    iota_np = np.broadcast_to(np.arange(P, dtype=np.float32), (P, P)).astype(BF16)
    shared = {
        "A1aug": np.asarray(A1aug), "A2aug": np.asarray(A2aug),
        "xbf": np.asarray(x_bf),
        "r1aug": np.asarray(r1aug), "r2aug": np.asarray(r2aug),
        "iota": np.asarray(iota_np),
    }
    EPC = WPC * U
    in_maps = []
    ones9 = np.ones((1, NPC), dtype=BF16)
    for c in range(NCORES):
        s = c * EPC
        m = dict(shared)
        m["attrT"] = attrT_all[:, s:s + EPC].copy()
        # [P, NT] with column (w*T + t) = edges [wU + t*128 : wU + (t+1)*128)
        m["srcw"] = np.ascontiguousarray(
            srcn_all[s:s + EPC].reshape(NT, P).T)
        m["dstw"] = np.ascontiguousarray(
            dstrel_all[s:s + EPC].reshape(NT, P).T)
        xTc = np.ascontiguousarray(x_bf[c * NPC:(c + 1) * NPC].T)
        m["xT9"] = np.concatenate([xTc, ones9], axis=0)
        in_maps.append(m)

    res = run_bass_kernel_spmd(nc, in_maps, list(range(NCORES)),
                               **kernel.run_kwargs)
    kernel.last_result = res
    out_pi = np.concatenate([res.results[c]["out"] for c in range(NCORES)], axis=0)
    return out_pi[perm]


kernel.run_kwargs = {}
kernel.last_result = None
